# revision 11
# baseline (speedup 1.0000x reference)
"""Trainium2 Bass kernel for nn_MinimumSpanningTree.

Contract: kernel(**inputs) takes the FULL inputs (guide_in [8, 64, 256, 256]
f32) and returns the FULL output (tree [8, 65535, 2] int32).

Strategy (data-parallel over batch, one image per NeuronCore):
  - Device (Bass, 8 cores SPMD): the memory-bound edge-weight build in fp16
    (empirically verified: fp16 rounding flips ~180/524k MST edges ->
    rel_err ~9e-3, inside the 2e-2 budget with 2x margin).
    Pipeline per image: DMA fp16 [64, 65536+pad] in 4 chunk-pair tiles
    [128, 8192+257] (partitions 0-63 = chunk t, 64-127 = chunk t+4);
    shifted subtract (row: +256, col: +1) split DVE/Pool; square split
    ACT/DVE/Pool; channel-reduce on PE as stationary-sq matmuls
    (lhsT = sq[:, 128b:128b+128], rhs = group-mask ones [128, 2]) ->
    PSUM [128, 2] f32 per block; evac PSUM->SBUF wall; one DMA out.
  - Boruvka MST per image (exactly the reference algorithm) on host +
    output assembly.

Self-contained: shapes/sharding hardcoded.
"""
import numpy as np

B, C, H, W = 8, 64, 256, 256
V = H * W
E_ROW = (H - 1) * W
E_COL = H * (W - 1)
E = E_ROW + E_COL
N_ROUNDS = 16

PAD = 260
TCOLS = 4096          # pixels per chunk
NT = 8                # tiles; tile t packs chunk t (parts 0-63) + chunk t+NT
NBLK = TCOLS // 128   # matmul blocks per tile per edge type

# Per-tile engine split (columns, multiples of 128), tuned against the
# TimelineSim cost model. Pool's sub_c range is squared by Pool itself so
# its chain is self-contained (no cross-engine stall).
GP0 = 2944            # sub_c/sq_c: DVE [0:GP0], Pool [GP0:TCOLS]
ACT0 = 2688           # sq_r and sq_c: ACT [0:ACT0] (one merged op), DVE rest

_compiled = None


def _build_program():
    import concourse.bacc as bacc
    import concourse.mybir as mybir
    from concourse import tile

    F32 = mybir.dt.float32
    F16 = mybir.dt.float16
    AL = mybir.AluOpType
    ACT = mybir.ActivationFunctionType

    nc = bacc.Bacc('TRN2', target_bir_lowering=False, debug=False, num_devices=8)
    d_fm = nc.dram_tensor("fm", [C, V + PAD], F16, kind="ExternalInput")
    # wall[po, t*128 + half*64 + 2*blk + g] = d_half[(t + NT*g)*TCOLS + blk*128 + po]
    o_w = nc.dram_tensor("w", [128, NT * 128], F32, kind="ExternalOutput")

    # output DMA groups (by tile): ship finished groups during the stream so
    # only the last tile's evac+DMA sits in the tail
    OUT_GROUPS = [(0, 3), (3, 6), (6, 7), (7, 8)]

    with tile.TileContext(nc) as tc:
        with tc.tile_pool(name="inp", bufs=3) as inp, \
             tc.tile_pool(name="dif", bufs=3) as dif, \
             tc.tile_pool(name="sqp", bufs=3) as sqp, \
             tc.tile_pool(name="cst", bufs=1) as cst, \
             tc.tile_pool(name="ps", bufs=1, space="PSUM") as psum:
            onesW = cst.tile([128, 2], F16)
            nc.vector.memset(onesW[:], 0.0)
            nc.vector.memset(onesW[0:64, 0:1], 1.0)
            nc.vector.memset(onesW[64:128, 1:2], 1.0)
            wall = cst.tile([128, NT * 128], F32)
            pw = psum.tile([128, NT * 128], F32)
            # preload the ACT Square table during the DMA head window
            dummy = cst.tile([128, 2], F16)
            nc.scalar.activation(dummy[:], onesW[:], ACT.Square)

            sq_tiles = {}
            out_dmas = []   # deferred to the end of the SP stream

            def emit_tile(t):
                x = inp.tile([128, TCOLS + 257], F16, tag="in")
                a0 = t * TCOLS
                b0 = (t + NT) * TCOLS
                # tile 0: split loads (non-overlapping) so compute starts sooner
                loads = [(0, 1281), (1281, TCOLS + 257)] if t == 0 else [(0, TCOLS + 257)]
                for s0, s1 in loads:
                    nc.sync.dma_start(x[0:64, s0:s1], d_fm[:, a0 + s0: a0 + s1])
                    nc.sync.dma_start(x[64:128, s0:s1], d_fm[:, b0 + s0: b0 + s1])
                splits = [(0, 1024), (1024, TCOLS)] if t == 0 else [(0, TCOLS)]

                # d: row diffs in cols [0:TCOLS], col diffs in [TCOLS:2*TCOLS]
                d = dif.tile([128, 2 * TCOLS], F16, tag="d")
                s = sqp.tile([128, 2 * TCOLS], F16, tag="s")
                sq_tiles[t] = s

                # subtracts: row (+256) all on DVE; col (+1) DVE head, Pool tail
                for s0, s1 in splits:
                    nc.vector.tensor_tensor(d[:, s0:s1], x[:, s0:s1],
                                            x[:, s0 + 256:s1 + 256], AL.subtract)
                nc.vector.tensor_tensor(d[:, TCOLS:TCOLS + GP0], x[:, 0:GP0],
                                        x[:, 1:GP0 + 1], AL.subtract)
                nc.gpsimd.tensor_tensor(d[:, TCOLS + GP0:2 * TCOLS], x[:, GP0:TCOLS],
                                        x[:, GP0 + 1:TCOLS + 1], AL.subtract)
                # squares: Pool squares its own range (self-contained chain);
                # ACT squares [0:ACT0] of both halves in ONE grouped-AP op
                nc.gpsimd.tensor_tensor(s[:, TCOLS + GP0:2 * TCOLS],
                                        d[:, TCOLS + GP0:2 * TCOLS],
                                        d[:, TCOLS + GP0:2 * TCOLS], AL.mult)
                if t == 0:
                    nc.scalar.activation(s[:, 0:1024], d[:, 0:1024], ACT.Square)
                    dg = d[:].rearrange("p (h c) -> p h c", h=2)[:, :, 1024:ACT0]
                    sg = s[:].rearrange("p (h c) -> p h c", h=2)[:, :, 1024:ACT0]
                    nc.scalar.activation(sg, dg, ACT.Square)
                    nc.scalar.activation(s[:, TCOLS:TCOLS + 1024],
                                         d[:, TCOLS:TCOLS + 1024], ACT.Square)
                else:
                    dg = d[:].rearrange("p (h c) -> p h c", h=2)[:, :, 0:ACT0]
                    sg = s[:].rearrange("p (h c) -> p h c", h=2)[:, :, 0:ACT0]
                    nc.scalar.activation(sg, dg, ACT.Square)
                nc.vector.tensor_tensor(s[:, ACT0:TCOLS], d[:, ACT0:TCOLS],
                                        d[:, ACT0:TCOLS], AL.mult)
                nc.vector.tensor_tensor(s[:, TCOLS + ACT0:TCOLS + GP0],
                                        d[:, TCOLS + ACT0:TCOLS + GP0],
                                        d[:, TCOLS + ACT0:TCOLS + GP0], AL.mult)

            def emit_matmuls(t):
                # channel reduce on PE: stationary sq block, moving group masks.
                # Order blocks so ranges that complete LAST come last (tail flush).
                s = sq_tiles.pop(t)
                base = 128 * t
                NA = ACT0 // 128
                NG = GP0 // 128
                order = ([("r", b) for b in range(NA, NBLK)] +          # DVE sq_r
                         [("r", b) for b in range(NA)] +                # ACT r half
                         [("c", b) for b in range(NA)] +                # ACT c half
                         [("c", b) for b in range(NA, NG)] +            # DVE sq_c
                         [("c", b) for b in range(NG, NBLK)])           # Pool
                for half, b in order:
                    off = 0 if half == "r" else 64
                    src0 = (0 if half == "r" else TCOLS) + 128 * b
                    nc.tensor.matmul(pw[:, base + off + 2 * b: base + off + 2 * b + 2],
                                     s[:, src0:src0 + 128], onesW[:],
                                     start=True, stop=True)

            done_groups = set()

            def emit_evacs(upto):
                # evac any fully-reduced groups; output DMAs deferred to the end
                for gi, (g0, g1) in enumerate(OUT_GROUPS):
                    if gi in done_groups or g1 > upto:
                        continue
                    done_groups.add(gi)
                    c0, c1 = 128 * g0, 128 * g1
                    h = (c0 + c1) // 2
                    if c1 - c0 > 128:
                        nc.vector.tensor_copy(wall[:, c0:h], pw[:, c0:h])
                        nc.scalar.activation(wall[:, h:c1], pw[:, h:c1], ACT.Copy)
                    else:
                        nc.vector.tensor_copy(wall[:, c0:c1], pw[:, c0:c1])
                    out_dmas.append((c0, c1))

            for t in range(NT):
                emit_tile(t)
                if t >= 1:
                    emit_matmuls(t - 1)   # one-tile lag: sems mostly satisfied
                emit_evacs(t - 1)
            emit_matmuls(NT - 1)
            emit_evacs(NT)
            # output DMAs last in the SP stream: they can't head-block inputs
            for c0, c1 in out_dmas:
                nc.sync.dma_start(o_w[:, c0:c1], wall[:, c0:c1])

    nc.compile()
    return nc


def _get_program():
    global _compiled
    if _compiled is None:
        _compiled = _build_program()
    return _compiled


def _edge_weights_device(guide_in):
    """Run on 8 cores; returns w [B, 2, V] f32: [b, 0] = d_row, [b, 1] = d_col."""
    from concourse.bass_utils import run_bass_kernel_spmd

    nc = _get_program()
    pad = np.zeros((C, PAD), np.float16)
    in_maps = []
    for b in range(B):
        fm = guide_in[b].reshape(C, V).astype(np.float16)
        in_maps.append({"fm": np.concatenate([fm, pad], axis=1)})
    res = run_bass_kernel_spmd(nc, in_maps, list(range(8)))

    out = np.empty((B, 2, V), np.float32)
    for b in range(B):
        wall = np.asarray(res.results[b]["w"])          # [128, 1024]
        a = wall.reshape(128, NT, 2, NBLK, 2)            # [po, t, half, blk, g]
        # pixel = (t + 4g)*8192 + blk*128 + po  ->  order [half, g, t, blk, po]
        out[b] = a.transpose(2, 4, 1, 3, 0).reshape(2, V)
    return out


def _build_index():
    raw = np.arange(V, dtype=np.int32).reshape(H, W)
    row_e = np.stack([raw[:-1, :], raw[1:, :]], axis=-1).reshape(-1, 2)
    col_e = np.stack([raw[:, :-1], raw[:, 1:]], axis=-1).reshape(-1, 2)
    return np.concatenate([row_e, col_e], axis=0)


def _scatter_min(target, keys, vals):
    order = np.argsort(keys, kind="stable")
    ks = keys[order]
    vs = vals[order]
    starts = np.flatnonzero(np.r_[True, ks[1:] != ks[:-1]])
    mins = np.minimum.reduceat(vs, starts)
    target[ks[starts]] = np.minimum(target[ks[starts]], mins)


def _mst_boruvka(u, v, w):
    """Exact port of the reference Boruvka (per image)."""
    eidx = np.arange(E, dtype=np.int64)
    vidx = np.arange(V, dtype=np.int64)
    INF = np.float32(np.inf)
    BIGE = E
    comp = vidx.copy()
    sel = np.zeros(E, dtype=bool)
    for _ in range(N_ROUNDS):
        cu, cv = comp[u], comp[v]
        active = cu != cv
        if not active.any():
            break
        wa = np.where(active, w, INF)
        minw = np.full(V, INF, np.float32)
        _scatter_min(minw, cu, wa)
        _scatter_min(minw, cv, wa)
        cand_u = np.where(active & (wa == minw[cu]), eidx, BIGE)
        cand_v = np.where(active & (wa == minw[cv]), eidx, BIGE)
        best = np.full(V, BIGE, np.int64)
        _scatter_min(best, cu, cand_u)
        _scatter_min(best, cv, cand_v)
        has = best < BIGE
        be = np.clip(best, 0, E - 1)
        cu_b, cv_b = comp[u[be]], comp[v[be]]
        parent = np.where(has, np.where(cu_b == vidx, cv_b, cu_b), vidx)
        pp = parent[parent]
        parent = np.where((pp == vidx) & (vidx < parent), vidx, parent)
        for _ in range(N_ROUNDS):
            parent = parent[parent]
        comp = parent[comp]
        sel[best[has]] = True
    return sel


def kernel(guide_in):
    guide_in = np.asarray(guide_in, dtype=np.float32)
    d = _edge_weights_device(guide_in)   # [B, 2, V]

    index = _build_index()
    u = index[:, 0].astype(np.int64)
    v = index[:, 1].astype(np.int64)
    trees = []
    for b in range(B):
        wr = d[b, 0, :E_ROW] + np.float32(1.0)
        wc = d[b, 1].reshape(H, W)[:, :W - 1].reshape(-1) + np.float32(1.0)
        w = np.concatenate([wr, wc]).astype(np.float32)
        sel = _mst_boruvka(u, v, w)
        eids = np.nonzero(sel)[0]
        if len(eids) != V - 1:
            eids = np.concatenate([eids, np.zeros(max(0, V - 1 - len(eids)), np.int64)])[:V - 1]
        trees.append(index[eids])
    return np.stack(trees).astype(np.int32)


# revision 13
# speedup vs baseline: 1.0175x; 1.0175x over previous
"""Trainium2 Bass kernel for nn_MinimumSpanningTree.

Contract: kernel(**inputs) takes the FULL inputs (guide_in [8, 64, 256, 256]
f32) and returns the FULL output (tree [8, 65535, 2] int32).

Strategy (data-parallel over batch, one image per NeuronCore):
  - Device (Bass, 8 cores SPMD): the memory-bound edge-weight build in fp16
    (empirically verified: fp16 rounding flips ~180/524k MST edges ->
    rel_err ~9e-3, inside the 2e-2 budget with 2x margin).
    Pipeline per image: DMA fp16 [64, 65536+pad] in 4 chunk-pair tiles
    [128, 8192+257] (partitions 0-63 = chunk t, 64-127 = chunk t+4);
    shifted subtract (row: +256, col: +1) split DVE/Pool; square split
    ACT/DVE/Pool; channel-reduce on PE as stationary-sq matmuls
    (lhsT = sq[:, 128b:128b+128], rhs = group-mask ones [128, 2]) ->
    PSUM [128, 2] f32 per block; evac PSUM->SBUF wall; one DMA out.
  - Boruvka MST per image (exactly the reference algorithm) on host +
    output assembly.

Self-contained: shapes/sharding hardcoded.
"""
import numpy as np

B, C, H, W = 8, 64, 256, 256
V = H * W
E_ROW = (H - 1) * W
E_COL = H * (W - 1)
E = E_ROW + E_COL
N_ROUNDS = 16

PAD = 260
TCOLS = 4096          # pixels per chunk
NT = 8                # tiles; tile t packs chunk t (parts 0-63) + chunk t+NT
NBLK = TCOLS // 128   # matmul blocks per tile per edge type

# Per-tile engine split (columns, multiples of 128), tuned against the
# TimelineSim cost model. Pool's sub_c range is squared by Pool itself so
# its chain is self-contained (no cross-engine stall).
GP0 = 2944            # sub_c/sq_c: DVE [0:GP0], Pool [GP0:TCOLS]
ACT0 = 2688           # sq_r and sq_c: ACT [0:ACT0] (one merged op), DVE rest

_compiled = None


def _build_program():
    import concourse.bacc as bacc
    import concourse.mybir as mybir
    from concourse import tile

    F32 = mybir.dt.float32
    F16 = mybir.dt.float16
    AL = mybir.AluOpType
    ACT = mybir.ActivationFunctionType

    nc = bacc.Bacc('TRN2', target_bir_lowering=False, debug=False, num_devices=8)
    d_fm = nc.dram_tensor("fm", [C, V + PAD], F16, kind="ExternalInput")
    # wall[po, t*128 + half*64 + 2*blk + g] = d_half[(t + NT*g)*TCOLS + blk*128 + po]
    o_w = nc.dram_tensor("w", [128, NT * 128], F32, kind="ExternalOutput")

    # output DMA groups (by tile): ship finished groups during the stream so
    # only the last tile's evac+DMA sits in the tail
    OUT_GROUPS = [(0, 3), (3, 6), (6, 7), (7, 8)]

    with tile.TileContext(nc) as tc:
        with tc.tile_pool(name="inp", bufs=3) as inp, \
             tc.tile_pool(name="dif", bufs=3) as dif, \
             tc.tile_pool(name="sqp", bufs=3) as sqp, \
             tc.tile_pool(name="cst", bufs=1) as cst, \
             tc.tile_pool(name="ps", bufs=1, space="PSUM") as psum:
            onesW = cst.tile([128, 2], F16)
            nc.vector.memset(onesW[:], 0.0)
            nc.vector.memset(onesW[0:64, 0:1], 1.0)
            nc.vector.memset(onesW[64:128, 1:2], 1.0)
            wall = cst.tile([128, NT * 128], F32)
            pw = psum.tile([128, NT * 128], F32)
            # preload the ACT Square table during the DMA head window
            dummy = cst.tile([128, 2], F16)
            nc.scalar.activation(dummy[:], onesW[:], ACT.Square)

            sq_tiles = {}
            out_dmas = []   # deferred to the end of the SP stream

            def emit_tile(t):
                x = inp.tile([128, TCOLS + 257], F16, tag="in")
                a0 = t * TCOLS
                b0 = (t + NT) * TCOLS
                # tile 0: split loads (non-overlapping) so compute starts sooner
                loads = [(0, 2305), (2305, TCOLS + 257)] if t == 0 else [(0, TCOLS + 257)]
                for s0, s1 in loads:
                    nc.sync.dma_start(x[0:64, s0:s1], d_fm[:, a0 + s0: a0 + s1])
                    nc.sync.dma_start(x[64:128, s0:s1], d_fm[:, b0 + s0: b0 + s1])
                splits = [(0, 2048), (2048, TCOLS)] if t == 0 else [(0, TCOLS)]

                # d: row diffs in cols [0:TCOLS], col diffs in [TCOLS:2*TCOLS]
                d = dif.tile([128, 2 * TCOLS], F16, tag="d")
                s = sqp.tile([128, 2 * TCOLS], F16, tag="s")
                sq_tiles[t] = s

                # subtracts: row (+256) all on DVE; col (+1) DVE head, Pool tail
                for s0, s1 in splits:
                    nc.vector.tensor_tensor(d[:, s0:s1], x[:, s0:s1],
                                            x[:, s0 + 256:s1 + 256], AL.subtract)
                nc.vector.tensor_tensor(d[:, TCOLS:TCOLS + GP0], x[:, 0:GP0],
                                        x[:, 1:GP0 + 1], AL.subtract)
                nc.gpsimd.tensor_tensor(d[:, TCOLS + GP0:2 * TCOLS], x[:, GP0:TCOLS],
                                        x[:, GP0 + 1:TCOLS + 1], AL.subtract)
                # squares: Pool squares its own range (self-contained chain);
                # ACT squares [0:ACT0] of both halves in ONE grouped-AP op
                nc.gpsimd.tensor_tensor(s[:, TCOLS + GP0:2 * TCOLS],
                                        d[:, TCOLS + GP0:2 * TCOLS],
                                        d[:, TCOLS + GP0:2 * TCOLS], AL.mult)
                if t == 0:
                    nc.scalar.activation(s[:, 0:2048], d[:, 0:2048], ACT.Square)
                    nc.scalar.activation(s[:, 2048:ACT0], d[:, 2048:ACT0], ACT.Square)
                    nc.scalar.activation(s[:, TCOLS:TCOLS + ACT0],
                                         d[:, TCOLS:TCOLS + ACT0], ACT.Square)
                else:
                    dg = d[:].rearrange("p (h c) -> p h c", h=2)[:, :, 0:ACT0]
                    sg = s[:].rearrange("p (h c) -> p h c", h=2)[:, :, 0:ACT0]
                    nc.scalar.activation(sg, dg, ACT.Square)
                nc.vector.tensor_tensor(s[:, ACT0:TCOLS], d[:, ACT0:TCOLS],
                                        d[:, ACT0:TCOLS], AL.mult)
                nc.vector.tensor_tensor(s[:, TCOLS + ACT0:TCOLS + GP0],
                                        d[:, TCOLS + ACT0:TCOLS + GP0],
                                        d[:, TCOLS + ACT0:TCOLS + GP0], AL.mult)

            def emit_matmuls(t):
                # channel reduce on PE: stationary sq block, moving group masks.
                # Order blocks so ranges that complete LAST come last (tail flush).
                s = sq_tiles.pop(t)
                base = 128 * t
                NA = ACT0 // 128
                NG = GP0 // 128
                order = ([("r", b) for b in range(NA, NBLK)] +          # DVE sq_r
                         [("r", b) for b in range(NA)] +                # ACT r half
                         [("c", b) for b in range(NA)] +                # ACT c half
                         [("c", b) for b in range(NA, NG)] +            # DVE sq_c
                         [("c", b) for b in range(NG, NBLK)])           # Pool
                for half, b in order:
                    off = 0 if half == "r" else 64
                    src0 = (0 if half == "r" else TCOLS) + 128 * b
                    nc.tensor.matmul(pw[:, base + off + 2 * b: base + off + 2 * b + 2],
                                     s[:, src0:src0 + 128], onesW[:],
                                     start=True, stop=True)

            done_groups = set()

            def emit_evacs(upto):
                # evac any fully-reduced groups; output DMAs deferred to the end
                for gi, (g0, g1) in enumerate(OUT_GROUPS):
                    if gi in done_groups or g1 > upto:
                        continue
                    done_groups.add(gi)
                    c0, c1 = 128 * g0, 128 * g1
                    h = (c0 + c1) // 2
                    if c1 - c0 > 128:
                        nc.vector.tensor_copy(wall[:, c0:h], pw[:, c0:h])
                        nc.scalar.activation(wall[:, h:c1], pw[:, h:c1], ACT.Copy)
                    else:
                        nc.vector.tensor_copy(wall[:, c0:c1], pw[:, c0:c1])
                    out_dmas.append((c0, c1))

            for t in range(NT):
                emit_tile(t)
                if t >= 1:
                    emit_matmuls(t - 1)   # one-tile lag: sems mostly satisfied
                emit_evacs(t - 1)
            emit_matmuls(NT - 1)
            emit_evacs(NT)
            # output DMAs last in the SP stream: they can't head-block inputs
            for c0, c1 in out_dmas:
                nc.sync.dma_start(o_w[:, c0:c1], wall[:, c0:c1])

    nc.compile()
    return nc


def _get_program():
    global _compiled
    if _compiled is None:
        _compiled = _build_program()
    return _compiled


def _edge_weights_device(guide_in):
    """Run on 8 cores; returns w [B, 2, V] f32: [b, 0] = d_row, [b, 1] = d_col."""
    from concourse.bass_utils import run_bass_kernel_spmd

    nc = _get_program()
    pad = np.zeros((C, PAD), np.float16)
    in_maps = []
    for b in range(B):
        fm = guide_in[b].reshape(C, V).astype(np.float16)
        in_maps.append({"fm": np.concatenate([fm, pad], axis=1)})
    res = run_bass_kernel_spmd(nc, in_maps, list(range(8)))

    out = np.empty((B, 2, V), np.float32)
    for b in range(B):
        wall = np.asarray(res.results[b]["w"])          # [128, 1024]
        a = wall.reshape(128, NT, 2, NBLK, 2)            # [po, t, half, blk, g]
        # pixel = (t + 4g)*8192 + blk*128 + po  ->  order [half, g, t, blk, po]
        out[b] = a.transpose(2, 4, 1, 3, 0).reshape(2, V)
    return out


def _build_index():
    raw = np.arange(V, dtype=np.int32).reshape(H, W)
    row_e = np.stack([raw[:-1, :], raw[1:, :]], axis=-1).reshape(-1, 2)
    col_e = np.stack([raw[:, :-1], raw[:, 1:]], axis=-1).reshape(-1, 2)
    return np.concatenate([row_e, col_e], axis=0)


def _scatter_min(target, keys, vals):
    order = np.argsort(keys, kind="stable")
    ks = keys[order]
    vs = vals[order]
    starts = np.flatnonzero(np.r_[True, ks[1:] != ks[:-1]])
    mins = np.minimum.reduceat(vs, starts)
    target[ks[starts]] = np.minimum(target[ks[starts]], mins)


def _mst_boruvka(u, v, w):
    """Exact port of the reference Boruvka (per image)."""
    eidx = np.arange(E, dtype=np.int64)
    vidx = np.arange(V, dtype=np.int64)
    INF = np.float32(np.inf)
    BIGE = E
    comp = vidx.copy()
    sel = np.zeros(E, dtype=bool)
    for _ in range(N_ROUNDS):
        cu, cv = comp[u], comp[v]
        active = cu != cv
        if not active.any():
            break
        wa = np.where(active, w, INF)
        minw = np.full(V, INF, np.float32)
        _scatter_min(minw, cu, wa)
        _scatter_min(minw, cv, wa)
        cand_u = np.where(active & (wa == minw[cu]), eidx, BIGE)
        cand_v = np.where(active & (wa == minw[cv]), eidx, BIGE)
        best = np.full(V, BIGE, np.int64)
        _scatter_min(best, cu, cand_u)
        _scatter_min(best, cv, cand_v)
        has = best < BIGE
        be = np.clip(best, 0, E - 1)
        cu_b, cv_b = comp[u[be]], comp[v[be]]
        parent = np.where(has, np.where(cu_b == vidx, cv_b, cu_b), vidx)
        pp = parent[parent]
        parent = np.where((pp == vidx) & (vidx < parent), vidx, parent)
        for _ in range(N_ROUNDS):
            parent = parent[parent]
        comp = parent[comp]
        sel[best[has]] = True
    return sel


def kernel(guide_in):
    guide_in = np.asarray(guide_in, dtype=np.float32)
    d = _edge_weights_device(guide_in)   # [B, 2, V]

    index = _build_index()
    u = index[:, 0].astype(np.int64)
    v = index[:, 1].astype(np.int64)
    trees = []
    for b in range(B):
        wr = d[b, 0, :E_ROW] + np.float32(1.0)
        wc = d[b, 1].reshape(H, W)[:, :W - 1].reshape(-1) + np.float32(1.0)
        w = np.concatenate([wr, wc]).astype(np.float32)
        sel = _mst_boruvka(u, v, w)
        eids = np.nonzero(sel)[0]
        if len(eids) != V - 1:
            eids = np.concatenate([eids, np.zeros(max(0, V - 1 - len(eids)), np.int64)])[:V - 1]
        trees.append(index[eids])
    return np.stack(trees).astype(np.int32)
